# revision 7
# baseline (speedup 1.0000x reference)
import os
import sys

import numpy as np

sys.path.insert(0, "/opt/trn_rl_repo")

import ml_dtypes
import concourse.bass as bass
from concourse import bacc
import concourse.mybir as mybir
import concourse.tile as tile
from concourse.bass_utils import run_bass_kernel_spmd

# Problem constants (hardcoded per contract)
B, L, N, H, HU = 512, 16, 10000, 128, 128
NCORES = 8
BL = B // NCORES            # 64 batch rows per core
T2 = 2 * L                  # 32 node/coord time steps
COLS = T2 * BL              # 2048 cols, t-major: col = t*BL + b
LCOLS = L * BL              # 1024 tau cols
NKT = 80                    # k-tiles (79 data + 1 zero pad)
NPAD = NKT * 128            # 10240
NBLK = 4                    # gemm column blocks of 512
NPAIR = NKT // 2            # 40 k-tile pairs per block DMA

F32 = mybir.dt.float32
BF16 = mybir.dt.bfloat16
NPBF = ml_dtypes.bfloat16

SIG = mybir.ActivationFunctionType.Sigmoid
TANH = mybir.ActivationFunctionType.Tanh
IDENT = mybir.ActivationFunctionType.Identity
MUL = mybir.AluOpType.mult
ADD = mybir.AluOpType.add

# bf16 packed constants, column offsets
C_WC = 0        # Wcoord.T      [2,128]
C_WTAU = 128    # Wtau.T        [1,128]
C_WX = 256      # (Wx2@Wx1).T   [2,128]
C_WRES = 384    # Wres.T        [1,128]
C_WE = 512      # (Wend2@Wend1).T [2,128]
C_W2 = 640      # head_W2       [128,7]
C_XIN = 647     # x.T           [2,64]
C_T0 = 711      # t0.T          [1,64]
C_END = 775     # end.T         [2,64]
C_TAU = 839     # tau           [1,1024]
C_COORDS = 1863 # coords        [2,2048]
C_ID = 3911     # identity      [128,128]
CPW = 4039

# f32 packed biases, column offsets
Z_BTAU = 0
Z_BX = 1
Z_BRES = 2
Z_BE = 3
Z_BG = 4        # col 4 + k*4 + slot (slot order i,f,o,g)
Z_B1 = 32       # [128,7]
Z_B2 = 39       # [1,7] on row 0
CBW = 46

SLOT2TORCH = [0, 1, 3, 2]   # slot i,f,o,g -> torch gate rows i,f,g,o

TK = [83, 19, 32, 32, 51, 64, 51]   # per-LSTM step counts

_prog_cache = {}


def _build_program():
    nc = bacc.Bacc()

    d_x = nc.declare_dram_parameter("xk", [NBLK, NPAIR, 128, 1024], BF16, isOutput=False)
    d_wn = nc.declare_dram_parameter("wn", [128, NKT, H], BF16, isOutput=False)
    d_cp = nc.declare_dram_parameter("cpack", [128, CPW], BF16, isOutput=False)
    d_cb = nc.declare_dram_parameter("cbias", [128, CBW], F32, isOutput=False)
    d_b4 = nc.declare_dram_parameter("b4", [4, 7 * 128], F32, isOutput=False)
    d_ind = nc.declare_dram_parameter("ind", [4, 256], F32, isOutput=False)
    d_wih = nc.declare_dram_parameter("wihT", [H, 7, 4 * H], BF16, isOutput=False)
    d_whh = nc.declare_dram_parameter("whhT", [H, 7, 4 * H], BF16, isOutput=False)
    d_w1 = nc.declare_dram_parameter("w1T", [H, 7, HU], BF16, isOutput=False)
    d_out = nc.declare_dram_parameter("out", [1, 7 * BL], F32, isOutput=True)

    with tile.TileContext(nc) as tc:
        with (
            tc.tile_pool(name="consts", bufs=1) as consts,
            tc.tile_pool(name="xpool", bufs=3) as xpool,
            tc.tile_pool(name="gsb", bufs=2) as gsb,
            tc.tile_pool(name="small", bufs=2) as small,
            tc.tile_pool(name="psG", bufs=1, space="PSUM") as psG,
            tc.tile_pool(name="psC", bufs=1, space="PSUM") as psC,
            tc.tile_pool(name="psK0", bufs=1, space="PSUM") as psK0,
            tc.tile_pool(name="psT1", bufs=1, space="PSUM") as psT1,
            tc.tile_pool(name="psT2", bufs=1, space="PSUM") as psT2,
        ):
            cp = consts.tile([128, CPW], BF16, tag="cp")
            nc.sync.dma_start(cp[:], d_cp[:])
            cb = consts.tile([128, CBW], F32, tag="cb")
            nc.sync.dma_start(cb[:], d_cb[:])
            b4 = consts.tile([4, 7 * 128], F32, tag="b4")
            nc.sync.dma_start(b4[:], d_b4[:])
            ind = consts.tile([4, 256], F32, tag="ind")
            nc.sync.dma_start(ind[:], d_ind[:])
            wih_sb = consts.tile([H, 7, 4 * H], BF16, tag="wih")
            nc.sync.dma_start(wih_sb[:], d_wih[:])
            whh_sb = consts.tile([H, 7, 4 * H], BF16, tag="whh")
            nc.sync.dma_start(whh_sb[:], d_whh[:])
            w1_sb = consts.tile([H, 7, HU], BF16, tag="w1")
            nc.sync.dma_start(w1_sb[:], d_w1[:])
            wn_sb = consts.tile([128, NKT, H], BF16, tag="wn")
            nc.sync.dma_start(wn_sb[:], d_wn[:])

            ident = cp[:, C_ID:C_ID + 128]

            # state: slots [k0,k2,k4,k5] and [k1,k3,k6]
            c_all = consts.tile([128, 4, BL], F32, tag="c_all")
            h_all = consts.tile([128, 4, BL], BF16, tag="h_all")
            c1_all = consts.tile([128, 3, BL], F32, tag="c1_all")
            h1_all = consts.tile([128, 3, BL], BF16, tag="h1_all")
            nc.vector.memset(c_all[:], 0.0)
            nc.vector.memset(h_all[:], 0.0)
            nc.vector.memset(c1_all[:], 0.0)
            nc.vector.memset(h1_all[:], 0.0)

            tauh_sb = consts.tile([H, LCOLS], BF16, tag="tauh")
            coordh_sb = consts.tile([H, COLS], BF16, tag="coordh")
            nodeh_sb = consts.tile([H, COLS], BF16, tag="nodeh")
            pre_sb = consts.tile([H, 3 * BL], BF16, tag="pre")

            # bulk ih+bias (bf16), gate-slot major per source
            ihk0_tau = consts.tile([128, 4, LCOLS], BF16, tag="ihk0_tau")
            ihk0_node = consts.tile([128, 4, COLS], BF16, tag="ihk0_node")
            ihk0_coord = consts.tile([128, 4, COLS], BF16, tag="ihk0_coord")
            ihk0_pre = consts.tile([128, 4, 3 * BL], BF16, tag="ihk0_pre")
            ihk2_node = consts.tile([128, 4, COLS], BF16, tag="ihk2_node")
            ihk4_tau = consts.tile([128, 4, LCOLS], BF16, tag="ihk4_tau")
            ihk4_node = consts.tile([128, 4, COLS], BF16, tag="ihk4_node")
            ihk4_pre = consts.tile([128, 4, 3 * BL], BF16, tag="ihk4_pre")
            ihk6_tau = consts.tile([128, 4, LCOLS], BF16, tag="ihk6_tau")
            ihk6_coord = consts.tile([128, 4, COLS], BF16, tag="ihk6_coord")
            ihk6_pre = consts.tile([128, 4, 3 * BL], BF16, tag="ihk6_pre")

            # ---- small projections ----
            pp = psC.tile([128, 512], F32, tag="c", name="pp_pre")
            nc.tensor.matmul(pp[:, 0:BL], cp[:2, C_WX:C_WX + 128],
                             cp[:2, C_XIN:C_XIN + BL], start=True, stop=False)
            nc.tensor.matmul(pp[:, BL:2 * BL], cp[:1, C_WRES:C_WRES + 128],
                             cp[:1, C_T0:C_T0 + BL], start=False, stop=False,
                             skip_group_check=True)
            nc.tensor.matmul(pp[:, 2 * BL:3 * BL], cp[:2, C_WE:C_WE + 128],
                             cp[:2, C_END:C_END + BL], start=False, stop=True,
                             skip_group_check=True)
            nc.scalar.activation(pre_sb[:, 0:BL], pp[:, 0:BL], TANH,
                                 bias=cb[:, Z_BX:Z_BX + 1])
            nc.scalar.activation(pre_sb[:, BL:2 * BL], pp[:, BL:2 * BL], TANH,
                                 bias=cb[:, Z_BRES:Z_BRES + 1])
            nc.scalar.activation(pre_sb[:, 2 * BL:3 * BL], pp[:, 2 * BL:3 * BL],
                                 IDENT, bias=cb[:, Z_BE:Z_BE + 1])

            for j in range(LCOLS // 512):
                ps = psC.tile([128, 512], F32, tag="c")
                nc.tensor.matmul(ps[:], cp[:1, C_WTAU:C_WTAU + 128],
                                 cp[:1, C_TAU + j * 512:C_TAU + (j + 1) * 512],
                                 start=True, stop=True)
                nc.scalar.activation(tauh_sb[:, j * 512:(j + 1) * 512], ps[:],
                                     TANH, bias=cb[:, Z_BTAU:Z_BTAU + 1])

            for j in range(COLS // 512):
                ps = psC.tile([128, 512], F32, tag="c")
                nc.tensor.matmul(ps[:], cp[:2, C_WC:C_WC + 128],
                                 cp[:2, C_COORDS + j * 512:C_COORDS + (j + 1) * 512],
                                 start=True, stop=True)
                nc.vector.tensor_copy(coordh_sb[:, j * 512:(j + 1) * 512], ps[:])

            # ---- bulk ih+bias chunks ----
            def bulk_chunk(k, ih_tile, src_ap, dst_lo, width):
                for s in range(4):
                    ps = psC.tile([128, 512], F32, tag="c")
                    nc.tensor.matmul(ps[:, 0:width],
                                     wih_sb[:, k, s * H:(s + 1) * H],
                                     src_ap, start=True, stop=True)
                    nc.vector.tensor_scalar_add(
                        ih_tile[:, s, dst_lo:dst_lo + width], ps[:, 0:width],
                        cb[:, Z_BG + k * 4 + s:Z_BG + k * 4 + s + 1])

            for j in range(2):
                bulk_chunk(0, ihk0_tau, tauh_sb[:, j * 512:(j + 1) * 512],
                           j * 512, 512)
                bulk_chunk(4, ihk4_tau, tauh_sb[:, j * 512:(j + 1) * 512],
                           j * 512, 512)
                bulk_chunk(6, ihk6_tau, tauh_sb[:, j * 512:(j + 1) * 512],
                           j * 512, 512)
            for j in range(4):
                bulk_chunk(0, ihk0_coord, coordh_sb[:, j * 512:(j + 1) * 512],
                           j * 512, 512)
                bulk_chunk(6, ihk6_coord, coordh_sb[:, j * 512:(j + 1) * 512],
                           j * 512, 512)
            bulk_chunk(0, ihk0_pre, pre_sb[:], 0, 192)
            bulk_chunk(4, ihk4_pre, pre_sb[:], 0, 192)
            bulk_chunk(6, ihk6_pre, pre_sb[:], 0, 192)

            # ---- node GEMM in 4 column blocks ----
            for blk in range(NBLK):
                gps = psG.tile([128, 512], F32, tag="g")
                for i in range(NPAIR):
                    xt = xpool.tile([128, 1024], BF16, tag="xt")
                    nc.sync.dma_start(xt[:], d_x[blk, i])
                    nc.tensor.matmul(gps[:], wn_sb[:, 2 * i],
                                     xt[:, 0:512], start=(i == 0),
                                     stop=False, skip_group_check=True)
                    nc.tensor.matmul(gps[:], wn_sb[:, 2 * i + 1],
                                     xt[:, 512:1024], start=False,
                                     stop=(i == NPAIR - 1),
                                     skip_group_check=True)
                nc.vector.tensor_copy(
                    nodeh_sb[:, blk * 512:(blk + 1) * 512], gps[:])
                bulk_chunk(0, ihk0_node,
                           nodeh_sb[:, blk * 512:(blk + 1) * 512],
                           blk * 512, 512)
                bulk_chunk(2, ihk2_node,
                           nodeh_sb[:, blk * 512:(blk + 1) * 512],
                           blk * 512, 512)
                bulk_chunk(4, ihk4_node,
                           nodeh_sb[:, blk * 512:(blk + 1) * 512],
                           blk * 512, 512)

            # ---- column getters ----
            def taucol(l):
                return tauh_sb[:, l * BL:(l + 1) * BL]

            def ncol(t):
                return nodeh_sb[:, t * BL:(t + 1) * BL]

            def ccol(t):
                return coordh_sb[:, t * BL:(t + 1) * BL]

            def precol(i):
                return pre_sb[:, i * BL:(i + 1) * BL]

            mk = lambda f, *a: (lambda: f(*a))
            seqs = {}
            pre_l = [mk(precol, 0), mk(precol, 1)]
            suf_l = [mk(precol, 2)]
            seqs[1] = pre_l + [mk(taucol, l) for l in range(L)] + suf_l
            seqs[3] = [mk(ccol, t) for t in range(T2)]
            seqs[6] = pre_l + [mk(f, t) for l in range(L)
                               for f, t in ((taucol, l), (ccol, 2 * l),
                                            (ccol, 2 * l + 1))] + suf_l
            seqs[4] = pre_l + [mk(f, t) for l in range(L)
                               for f, t in ((taucol, l), (ncol, 2 * l),
                                            (ncol, 2 * l + 1))] + suf_l
            seqs[5] = [mk(f, t) for l in range(L)
                       for f, t in ((ncol, 2 * l), (ccol, 2 * l),
                                    (ncol, 2 * l + 1), (ccol, 2 * l + 1))]

            def k4_src(t):
                if t == 0:
                    return ihk4_pre, 0
                if t == 1:
                    return ihk4_pre, 1
                if t == 50:
                    return ihk4_pre, 2
                l, r = divmod(t - 2, 3)
                if r == 0:
                    return ihk4_tau, l
                return ihk4_node, 2 * l + (1 if r == 2 else 0)

            def k6_src(t):
                if t == 0:
                    return ihk6_pre, 0
                if t == 1:
                    return ihk6_pre, 1
                if t == 50:
                    return ihk6_pre, 2
                l, r = divmod(t - 2, 3)
                if r == 0:
                    return ihk6_tau, l
                return ihk6_coord, 2 * l + (1 if r == 2 else 0)

            def k2_src(t):
                return ihk2_node, t

            def k0_src(t):
                if t == 0:
                    return ihk0_pre, 0
                if t == 1:
                    return ihk0_pre, 1
                if t == 82:
                    return ihk0_pre, 2
                l, r = divmod(t - 2, 5)
                if r == 0:
                    return ihk0_tau, l
                if r in (1, 3):
                    return ihk0_node, 2 * l + (1 if r == 3 else 0)
                return ihk0_coord, 2 * l + (1 if r == 4 else 0)

            # ---- LSTM ticks (breadth-first stages per round) ----
            TWO = 2.0
            NEG1 = -1.0

            def mm_member(ps3, sl, k, kind, t, hT, s):
                mode, arg = kind
                if mode == "ident":
                    src_t, idx = arg(t)
                    nc.tensor.matmul(ps3[:, sl, 0:256], ident,
                                     src_t[:, :, idx * BL:(idx + 1) * BL],
                                     start=True, stop=False,
                                     skip_group_check=True)
                else:
                    nc.tensor.matmul(ps3[:, sl, 0:256],
                                     b4[:, k * 128:(k + 1) * 128], ind[:],
                                     start=True, stop=False,
                                     skip_group_check=True)
                    xc = arg[t]()
                    for g in range(4):
                        nc.tensor.matmul(ps3[:, sl, g * 64:(g + 1) * 64],
                                         wih_sb[:, k, g * H:(g + 1) * H],
                                         xc, start=False, stop=False,
                                         skip_group_check=True)
                for g in range(4):
                    nc.tensor.matmul(ps3[:, sl, g * 64:(g + 1) * 64],
                                     whh_sb[:, k, g * H:(g + 1) * H],
                                     hT[:, s], start=False, stop=(g == 3),
                                     skip_group_check=True)

            class Group:
                pass

            g_k0 = Group()
            g_k0.members = [(0, 0, ("ident", k0_src))]
            g_k0.pool, g_k0.tag, g_k0.nslot, g_k0.soff = psK0, "k0ps", 1, 0
            g_k0.cT, g_k0.hT = c_all, h_all
            g_k0.gtag, g_k0.T = "gk0", 83

            g_t2 = Group()
            g_t2.members = [(2, 1, ("ident", k2_src)), (4, 2, ("ident", k4_src)),
                            (5, 3, ("inloop", seqs[5]))]
            g_t2.pool, g_t2.tag, g_t2.nslot, g_t2.soff = psT2, "ps2", 3, 1
            g_t2.cT, g_t2.hT = c_all, h_all
            g_t2.gtag, g_t2.T = "g3", 64

            g_t1 = Group()
            g_t1.members = [(3, 1, ("inloop", seqs[3])), (6, 2, ("ident", k6_src))]
            g_t1.pool, g_t1.tag, g_t1.nslot, g_t1.soff = psT1, "ps1", 2, 1
            g_t1.cT, g_t1.hT = c1_all, h1_all
            g_t1.gtag, g_t1.T = "g1", 51

            g_k1 = Group()
            g_k1.members = [(1, 0, ("inloop", seqs[1]))]
            g_k1.pool, g_k1.tag, g_k1.nslot, g_k1.soff = psK0, "k0ps", 1, 0
            g_k1.cT, g_k1.hT = c1_all, h1_all
            g_k1.gtag, g_k1.T = "gk0", 19

            def round_groups(t, groups):
                """Breadth-first emission of one tick across groups."""
                st = []
                for G in groups:
                    act = [(k, s, kind) for (k, s, kind) in G.members
                           if t < TK[k]]
                    if not act:
                        continue
                    lo, hi = act[0][1], act[-1][1] + 1
                    plo, phi = lo - G.soff, hi - G.soff
                    st.append((G, act, lo, hi, plo, phi))
                # stage 1: all matmuls
                for G, act, lo, hi, plo, phi in st:
                    G.ps = G.pool.tile([128, G.nslot, 512], F32, tag=G.tag,
                                       name=f"{G.tag}_{t}")
                for G, act, lo, hi, plo, phi in st:
                    for k, s, kind in act:
                        mm_member(G.ps, s - G.soff, k, kind, t, G.hT, s)
                # stage 2: fused sigmoid over all 4 gates (g-gate uses 2sig)
                for G, act, lo, hi, plo, phi in st:
                    G.gt = gsb.tile([128, G.nslot, 256], F32, tag=G.gtag)
                    nc.scalar.activation(G.gt[:, plo:phi, :],
                                         G.ps[:, plo:phi, 0:256], SIG)
                # stage 3: s' = 2*s_g - 1  (tanh fix-up)
                for G, act, lo, hi, plo, phi in st:
                    G.sp = small.tile([128, G.nslot, 64], F32, tag=G.gtag + "sp")
                    nc.vector.tensor_scalar(G.sp[:, plo:phi],
                                            G.gt[:, plo:phi, 192:256],
                                            TWO, NEG1, MUL, ADD)
                # stage 4: ig
                for G, act, lo, hi, plo, phi in st:
                    G.ig = small.tile([128, G.nslot, 64], F32, tag=G.gtag + "ig")
                    nc.vector.tensor_tensor(G.ig[:, plo:phi],
                                            G.gt[:, plo:phi, 0:64],
                                            G.sp[:, plo:phi], MUL)
                # stage 5: fc
                for G, act, lo, hi, plo, phi in st:
                    G.fc = small.tile([128, G.nslot, 64], F32, tag=G.gtag + "fc")
                    nc.vector.tensor_tensor(G.fc[:, plo:phi],
                                            G.gt[:, plo:phi, 64:128],
                                            G.cT[:, lo:hi], MUL)
                # stage 6: c = fc + ig
                for G, act, lo, hi, plo, phi in st:
                    nc.vector.tensor_tensor(G.cT[:, lo:hi], G.fc[:, plo:phi],
                                            G.ig[:, plo:phi], ADD)
                # stage 7: tanh(c)
                for G, act, lo, hi, plo, phi in st:
                    G.th = small.tile([128, G.nslot, 64], F32, tag=G.gtag + "th")
                    nc.scalar.activation(G.th[:, plo:phi], G.cT[:, lo:hi], TANH)
                # stage 8: h = o * tanh(c)
                for G, act, lo, hi, plo, phi in st:
                    nc.vector.tensor_tensor(G.hT[:, lo:hi],
                                            G.gt[:, plo:phi, 128:192],
                                            G.th[:, plo:phi], MUL)

            for t in range(19):
                round_groups(t, [g_k1])
            for t in range(83):
                round_groups(t, [g_k0, g_t2, g_t1])

            # ---- heads ----
            out_sb = consts.tile([1, 7 * BL], F32, tag="outsb")
            hmap = {0: (h_all, 0), 2: (h_all, 1), 4: (h_all, 2), 5: (h_all, 3),
                    1: (h1_all, 0), 3: (h1_all, 1), 6: (h1_all, 2)}
            for k in range(7):
                ht, s = hmap[k]
                hp = psK0.tile([128, 256], F32, tag="k0ps", name=f"hp{k}")
                nc.tensor.matmul(hp[:, 0:BL], w1_sb[:, k], ht[:, s],
                                 start=True, stop=False, skip_group_check=True)
                z1 = small.tile([128, BL], BF16, tag="z1")
                nc.scalar.activation(z1[:], hp[:, 0:BL], TANH,
                                     bias=cb[:, Z_B1 + k:Z_B1 + k + 1])
                nc.tensor.matmul(hp[0:1, BL:2 * BL], cp[:, C_W2 + k:C_W2 + k + 1],
                                 z1[:], start=True, stop=True,
                                 skip_group_check=True)
                nc.scalar.activation(out_sb[:, k * BL:(k + 1) * BL],
                                     hp[0:1, BL:2 * BL], IDENT,
                                     bias=cb[0:1, Z_B2 + k:Z_B2 + k + 1])

            nc.sync.dma_start(d_out[:], out_sb[:])

    nc.finalize()
    return nc


def _get_program():
    if "nc" not in _prog_cache:
        _prog_cache["nc"] = _build_program()
    return _prog_cache["nc"]


def _pack_constants(inp):
    cpk = np.zeros((128, CPW), NPBF)
    cbk = np.zeros((128, CBW), np.float32)

    def put(dst, c, arr):
        dst[:arr.shape[0], c:c + arr.shape[1]] = arr

    put(cpk, C_WC, inp["Wcoord"].T)
    put(cpk, C_WTAU, inp["Wtau"].T)
    put(cpk, C_WX, (inp["Wx2"] @ inp["Wx1"]).T)
    put(cpk, C_WRES, inp["Wres"].T)
    put(cpk, C_WE, (inp["Wend2"] @ inp["Wend1"]).T)
    put(cpk, C_W2, inp["head_W2"].reshape(7, HU).T)
    put(cpk, C_ID, np.eye(128, dtype=NPBF))
    put(cbk, Z_BTAU, inp["btau"][:, None])
    put(cbk, Z_BX, inp["bx2"][:, None])
    put(cbk, Z_BRES, inp["bres"][:, None])
    put(cbk, Z_BE, inp["bend2"][:, None])
    bsum = (inp["lstm_bih"] + inp["lstm_bhh"]).reshape(7, 4, H)
    GS = [1.0, 1.0, 1.0, 2.0]   # g-gate doubled for the 2*sigmoid trick
    for k in range(7):
        for s, tg in enumerate(SLOT2TORCH):
            cbk[:, Z_BG + k * 4 + s] = bsum[k, tg] * GS[s]
    put(cbk, Z_B1, inp["head_b1"].T)
    cbk[0, Z_B2:Z_B2 + 7] = inp["head_b2"].reshape(7)

    b4 = np.zeros((4, 7 * 128), np.float32)
    for k in range(7):
        for s, tg in enumerate(SLOT2TORCH):
            b4[s, k * 128:(k + 1) * 128] = bsum[k, tg] * GS[s]
    indm = np.zeros((4, 256), np.float32)
    for s in range(4):
        indm[s, s * 64:(s + 1) * 64] = 1.0
    return cpk, cbk, b4, indm


def _make_in_maps(inp):
    node = inp["node_inputs"]
    coords = inp["coords"]
    tau = inp["tau_inputs"]
    x = inp["x"]
    t0 = inp["t0_res"]
    end = inp["end"]

    wn = np.zeros((NPAD, H), NPBF)
    wn[:N] = inp["Wnode"].T
    wn_dev = np.ascontiguousarray(wn.reshape(NKT, 128, H).transpose(1, 0, 2))

    def slot_major(w):  # [7, 4H, H] -> [H, 7, 4H] slots i,f,o,g; g-gate doubled
        w4 = w.reshape(7, 4, H, H)[:, SLOT2TORCH].copy()
        w4[:, 3] *= 2.0   # tanh(z) = 2*sigmoid(2z) - 1
        return np.ascontiguousarray(
            w4.reshape(7, 4 * H, H).transpose(2, 0, 1).astype(NPBF))

    wih = slot_major(inp["lstm_Wih"])
    whh = slot_major(inp["lstm_Whh"])
    w1 = np.ascontiguousarray(inp["head_W1"].transpose(2, 0, 1).astype(NPBF))

    cpk_base, cbk, b4, indm = _pack_constants(inp)

    in_maps = []
    for c in range(NCORES):
        sl = slice(c * BL, (c + 1) * BL)
        Xn = np.zeros((NPAD, COLS), NPBF)
        Xn[:N] = node[sl].transpose(2, 1, 0).reshape(N, COLS)
        xk = np.ascontiguousarray(
            Xn.reshape(NPAIR, 2, 128, NBLK, 512)
            .transpose(3, 0, 2, 1, 4)
            .reshape(NBLK, NPAIR, 128, 1024))
        cpk = cpk_base.copy()
        cpk[:2, C_XIN:C_XIN + BL] = x[sl].T
        cpk[:1, C_T0:C_T0 + BL] = t0[sl].T
        cpk[:2, C_END:C_END + BL] = end[sl].T
        cpk[:1, C_TAU:C_TAU + LCOLS] = tau[sl].transpose(2, 1, 0).reshape(1, LCOLS)
        cpk[:2, C_COORDS:C_COORDS + COLS] = coords[sl].transpose(2, 1, 0).reshape(2, COLS)
        in_maps.append(dict(
            xk=xk, wn=wn_dev, cpack=cpk, cbias=cbk, b4=b4, ind=indm,
            wihT=wih, whhT=whh, w1T=w1,
        ))
    return in_maps


def kernel(**inputs):
    inp = {k: np.asarray(v, dtype=np.float32) for k, v in inputs.items()}
    in_maps = _make_in_maps(inp)
    nc = _get_program()
    extra = {}
    if os.environ.get("BASS_TMPDIR"):
        extra["tmpdir"] = os.environ["BASS_TMPDIR"]
    res = run_bass_kernel_spmd(nc, in_maps, core_ids=list(range(NCORES)), **extra)
    if res.exec_time_ns is not None:
        print(f"HW exec time: {res.exec_time_ns} ns")
    if res.instructions_and_trace is not None:
        print(f"trace path: {res.instructions_and_trace[1]}")

    outs = [r["out"].reshape(7, BL) for r in res.results]
    full = np.concatenate(outs, axis=1)      # [7, B]
    return tuple(full[k][:, None].astype(np.float32) for k in range(7))
